# revision 1
# baseline (speedup 1.0000x reference)
"""HQQ 1-bit quantized linear (out = x @ dequant(W).T + bias) on 8 Trainium2
NeuronCores.

Sharding: 2D tensor-parallel. x rows (M=8192) split in 2 halves, out_features
(4096) split in 4 columns -> 8 cores, each computing a [4096, 1024] output
shard with the full K=4096 contraction:
    core c: rows [4096*(c//4) : ...], out cols [1024*(c%4) : ...]

Per core the device kernel:
  - dequantizes its W shard on-chip: bit-extract via DVE shift/and from the
    packed bytes, then per-group affine (B*scale - zero*scale) into a resident
    bf16 weight tile [K=4096, O=1024] (transposed layout for the PE),
  - casts its x shard to bf16 on-chip,
  - accumulates out = bias + x @ W_hat.T on the tensor engine in fp32 PSUM.

Host-side work is layout-only: transpose/permute/replicate/slice + int16
container cast for the packed bytes (values 0..255 preserved exactly).
"""

import sys

for _p in ("/opt/trn_rl_repo", "/root/.axon_site/_ro/trn_rl_repo"):
    if _p not in sys.path:
        sys.path.append(_p)

import numpy as np

P = 128
MM_N = 512
NBITS_PER_BYTE = 8
GROUP_SIZE = 64
M_FULL, K_IN, O_FULL = 8192, 4096, 4096
M_SPLIT, O_SPLIT = 2, 4          # 2 x 4 = 8 cores
M_SH, O_SH = M_FULL // M_SPLIT, O_FULL // O_SPLIT
N_CORES = 8

_compiled = {}


def _build_nc(repeat=1, xcast_act=False):
    import concourse.bacc as bacc
    import concourse.mybir as mybir
    import concourse.tile as tile

    f32 = mybir.dt.float32
    bf16 = mybir.dt.bfloat16
    i16 = mybir.dt.int16

    PB = K_IN // NBITS_PER_BYTE   # 512 bytes per row
    N_KT = K_IN // P              # 32 k-tiles
    N_V = PB // P                 # 4 byte-tiles
    N_MT = M_SH // P              # 32 m-tiles
    OC = MM_N
    N_OC = O_SH // OC             # 2 o-chunks

    nc = bacc.Bacc("TRN2", target_bir_lowering=False, debug=False,
                   num_devices=N_CORES)

    xt_d = nc.dram_tensor("xt", [K_IN, M_SH], f32, kind="ExternalInput")
    wpt_d = nc.dram_tensor("wpt", [PB, O_SH], i16, kind="ExternalInput")
    sexp_d = nc.dram_tensor("sexp", [PB, O_SH], f32, kind="ExternalInput")
    zexp_d = nc.dram_tensor("zexp", [PB, O_SH], f32, kind="ExternalInput")
    bias_d = nc.dram_tensor("bias", [1, O_SH], f32, kind="ExternalInput")
    out_d = nc.dram_tensor("out", [M_SH, O_SH], f32, kind="ExternalOutput")

    with tile.TileContext(nc) as tc:
        with tc.tile_pool(name="fixed", bufs=1) as fixed, \
             tc.tile_pool(name="setup", bufs=1) as setup, \
             tc.tile_pool(name="deq", bufs=3) as deq, \
             tc.tile_pool(name="xtf", bufs=3) as xtf_pool, \
             tc.tile_pool(name="xtb", bufs=4) as xtb_pool, \
             tc.tile_pool(name="outp", bufs=3) as out_pool, \
             tc.tile_pool(name="psum", bufs=8, space="PSUM") as psum_pool:

            # constants
            ones_b = fixed.tile([1, P], bf16, tag="ones")
            nc.vector.memset(ones_b[:1, :], 1.0)
            bias_f = setup.tile([1, O_SH], f32, tag="biasf")
            nc.sync.dma_start(bias_f[:1, :], bias_d[:, :])
            bias_b = fixed.tile([1, O_SH], bf16, tag="biasb")
            nc.vector.tensor_copy(bias_b[:1, :], bias_f[:1, :])

            # per byte-tile group coefficients: s = scale, nzs = -zero*scale
            s_b, nzs_b = [], []
            for v in range(N_V):
                s_f = setup.tile([P, O_SH], f32, tag="sf", name="s_f")
                z_f = setup.tile([P, O_SH], f32, tag="zf", name="z_f")
                nc.sync.dma_start(s_f[:], sexp_d[v * P:(v + 1) * P, :])
                nc.sync.dma_start(z_f[:], zexp_d[v * P:(v + 1) * P, :])
                s_v = fixed.tile([P, O_SH], bf16, tag=f"s_{v}", name=f"s_{v}")
                nzs_v = fixed.tile([P, O_SH], bf16, tag=f"nzs_{v}", name=f"nzs_{v}")
                nc.vector.tensor_copy(s_v[:], s_f[:])
                nc.vector.scalar_tensor_tensor(
                    nzs_v[:], z_f[:], -1.0, s_f[:],
                    mybir.AluOpType.mult, mybir.AluOpType.mult)
                s_b.append(s_v)
                nzs_b.append(nzs_v)

            # packed weights
            wpt_sb = []
            for v in range(N_V):
                w_v = fixed.tile([P, O_SH], i16, tag=f"wpt_{v}", name=f"wpt_{v}")
                nc.sync.dma_start(w_v[:], wpt_d[v * P:(v + 1) * P, :])
                wpt_sb.append(w_v)

            # dequantize all k-tiles into resident bf16 WT [128, 32, 1024]
            WT = fixed.tile([P, N_KT, O_SH], bf16, tag="WT")
            for t in range(N_KT):
                u, v = t // N_V, t % N_V
                # bitVec ops cannot cast: keep shift/and int16 -> int16
                B_t = deq.tile([P, O_SH], i16, tag="B", name="B_t")
                nc.vector.tensor_scalar(
                    B_t[:], wpt_sb[v][:], u, 1,
                    mybir.AluOpType.logical_shift_right,
                    mybir.AluOpType.bitwise_and)
                # cast + scale in one fused op: (B * 1.0) * s  -> bf16
                bs_t = deq.tile([P, O_SH], bf16, tag="bs", name="bs_t")
                nc.vector.scalar_tensor_tensor(
                    bs_t[:], B_t[:], 1.0, s_b[v][:],
                    mybir.AluOpType.mult, mybir.AluOpType.mult)
                nc.vector.tensor_tensor(WT[:, t, :], bs_t[:], nzs_b[v][:],
                                        mybir.AluOpType.add)

            def load_cast(mi):
                xt_f = xtf_pool.tile([P, N_KT, P], f32, tag="xtf", name="xt_f")
                nc.sync.dma_start(
                    xt_f[:],
                    xt_d[:, mi * P:(mi + 1) * P].rearrange("(t p) m -> p t m", p=P))
                xt_b = xtb_pool.tile([P, N_KT, P], bf16, tag="xtb", name="xt_b")
                if xcast_act:
                    nc.scalar.copy(xt_b[:], xt_f[:])
                else:
                    nc.vector.tensor_copy(xt_b[:], xt_f[:])
                return xt_b

            def drain(ps, mi, oc):
                out_t = out_pool.tile([P, OC], f32, tag="out", name="out_t")
                nc.scalar.copy(out_t[:], ps[:])
                nc.sync.dma_start(
                    out_d[mi * P:(mi + 1) * P, oc * OC:(oc + 1) * OC], out_t[:])

            # First 4 m-tiles k-outer across all 8 PSUM banks: the PE consumes
            # each WT[t] right as dequant produces it instead of idling through
            # the whole dequant phase.
            FB = min(4, N_MT)
            for rep in range(repeat):
                xb0 = [load_cast(mi) for mi in range(FB)]
                pss = []
                for mi in range(FB):
                    for oc in range(N_OC):
                        ps = psum_pool.tile([P, OC], f32, tag="ps", name="ps")
                        nc.tensor.matmul(ps[:], ones_b[:1, :],
                                         bias_b[:1, oc * OC:(oc + 1) * OC],
                                         start=True, stop=False)
                        pss.append(ps)
                for t in range(N_KT):
                    for mi in range(FB):
                        for oc in range(N_OC):
                            nc.tensor.matmul(
                                pss[mi * N_OC + oc][:], xb0[mi][:, t, :],
                                WT[:, t, oc * OC:(oc + 1) * OC],
                                start=False, stop=(t == N_KT - 1))
                for mi in range(FB):
                    for oc in range(N_OC):
                        drain(pss[mi * N_OC + oc], mi, oc)

                # steady loop, t-outer / oc-inner so each stationary xt_b[t]
                # is reused for both o-chunks (halves LDWEIGHTS traffic)
                for mi in range(FB, N_MT):
                    xt_b = load_cast(mi)
                    pso = []
                    for oc in range(N_OC):
                        ps = psum_pool.tile([P, OC], f32, tag="ps", name="ps")
                        nc.tensor.matmul(ps[:], ones_b[:1, :],
                                         bias_b[:1, oc * OC:(oc + 1) * OC],
                                         start=True, stop=False)
                        pso.append(ps)
                    for t in range(N_KT):
                        for oc in range(N_OC):
                            nc.tensor.matmul(
                                pso[oc][:], xt_b[:, t, :],
                                WT[:, t, oc * OC:(oc + 1) * OC],
                                start=False, stop=(t == N_KT - 1))
                    for oc in range(N_OC):
                        drain(pso[oc], mi, oc)
    nc.compile()
    return nc


def _get_nc(**kw):
    key = tuple(sorted(kw.items()))
    if key not in _compiled:
        _compiled[key] = _build_nc(**kw)
    return _compiled[key]


def _host_prep(x, W_packed, scale, zero, bias):
    """Layout-only prep of per-core input maps."""
    PB = K_IN // NBITS_PER_BYTE
    x = np.asarray(x, dtype=np.float32)
    W_packed = np.asarray(W_packed)
    scale2d = np.asarray(scale, dtype=np.float32).reshape(O_FULL, K_IN // GROUP_SIZE)
    zero2d = np.asarray(zero, dtype=np.float32).reshape(O_FULL, K_IN // GROUP_SIZE)
    bias = np.asarray(bias, dtype=np.float32)

    # bit-plane-major permuted transpose of x halves:
    # xt[k*PB + p, m] = x[m, 8p + k]
    xt_half = []
    for h in range(M_SPLIT):
        xs = x[h * M_SH:(h + 1) * M_SH]                       # [M_SH, K_IN]
        xt = xs.T.reshape(PB, NBITS_PER_BYTE, M_SH)
        xt = np.ascontiguousarray(
            xt.transpose(1, 0, 2).reshape(K_IN, M_SH))
        xt_half.append(xt)

    in_maps = []
    for c in range(N_CORES):
        h, q = divmod(c, O_SPLIT)
        osl = slice(q * O_SH, (q + 1) * O_SH)
        wpt = np.ascontiguousarray(W_packed[osl].T.astype(np.int16))   # [PB, O_SH]
        sexp = np.ascontiguousarray(np.repeat(scale2d[osl].T, NBITS_PER_BYTE, axis=0))
        zexp = np.ascontiguousarray(np.repeat(zero2d[osl].T, NBITS_PER_BYTE, axis=0))
        in_maps.append(dict(
            xt=xt_half[h], wpt=wpt, sexp=sexp, zexp=zexp,
            bias=np.ascontiguousarray(bias[None, osl]),
        ))
    return in_maps


def run_sharded(x, W_packed, scale, zero, bias, trace=False, **run_kwargs):
    """Compile (cached), run on 8 cores, return (full_out, BassKernelResults)."""
    from concourse.bass_utils import run_bass_kernel_spmd

    nc = _get_nc()
    in_maps = _host_prep(x, W_packed, scale, zero, bias)
    res = run_bass_kernel_spmd(nc, in_maps, core_ids=list(range(N_CORES)),
                               trace=trace, **run_kwargs)
    out = np.empty((M_FULL, O_FULL), dtype=np.float32)
    for c in range(N_CORES):
        h, q = divmod(c, O_SPLIT)
        out[h * M_SH:(h + 1) * M_SH, q * O_SH:(q + 1) * O_SH] = \
            res.results[c]["out"]
    return out, res


def kernel(x, W_packed, scale, zero, bias):
    out, _ = run_sharded(x, W_packed, scale, zero, bias)
    return out



# revision 5
# speedup vs baseline: 1.0578x; 1.0578x over previous
"""HQQ 1-bit quantized linear (out = x @ dequant(W).T + bias) on 8 Trainium2
NeuronCores.

Sharding: 8-way row-parallel. x rows (M=8192) split into 8 shards of 1024;
every core computes its [1024, 4096] output slab against the FULL weight
matrix (K=4096 contraction, O=4096 out features). This reads x from HBM
exactly once across the 8 cores (16 MiB/core vs 64 MiB/core for the 2x4
tensor-parallel split), so DMA stays far below the PE roofline.

Per core the device kernel:
  - keeps the x shard resident in SBUF as bf16 [128, 32 kt, 1024] (cast from
    the f32 DMA),
  - streams the packed weights + per-group affine coefficients per o-chunk of
    512 out-features, dequantizing on DVE (shift/and bit-extract, then
    B*s + (-z*s)) into a double-buffered bf16 W tile [128, 32 kt, 512],
  - runs the 32-k-tile accumulation per (m-tile, o-chunk) on the tensor
    engine into fp32 PSUM (8 banks round-robin),
  - drains PSUM via DVE with a fused bias add (bias pre-broadcast across
    partitions once at setup via a rank-1 ones x bias matmul).

Host-side work is layout/packing only: transpose/permute/replicate/slice,
int16 container cast for the packed bytes, bf16 cast + per-group (-z*s)
product for the tiny [4096, 64] coefficient arrays.
"""

import sys

for _p in ("/opt/trn_rl_repo", "/root/.axon_site/_ro/trn_rl_repo"):
    if _p not in sys.path:
        sys.path.append(_p)

import numpy as np

P = 128
OC = 512                      # out-feature chunk per dequant/matmul round
NBITS_PER_BYTE = 8
GROUP_SIZE = 64
M_FULL, K_IN, O_FULL = 8192, 4096, 4096
N_CORES = 8
M_SH = M_FULL // N_CORES      # 1024 rows per core

_compiled = {}


def _build_nc():
    import concourse.bacc as bacc
    import concourse.mybir as mybir
    import concourse.tile as tile

    f32 = mybir.dt.float32
    bf16 = mybir.dt.bfloat16
    i16 = mybir.dt.int16

    PB = K_IN // NBITS_PER_BYTE   # 512 packed-byte rows
    N_KT = K_IN // P              # 32 k-tiles
    N_V = PB // P                 # 4 byte-tiles
    N_MT = M_SH // P              # 8 m-tiles
    N_OC = O_FULL // OC           # 8 o-chunks

    nc = bacc.Bacc("TRN2", target_bir_lowering=False, debug=False,
                   num_devices=N_CORES)

    xt_d = nc.dram_tensor("xt", [K_IN, M_SH], f32, kind="ExternalInput")
    wpt_d = nc.dram_tensor("wpt", [PB, O_FULL], i16, kind="ExternalInput")
    sexp_d = nc.dram_tensor("sexp", [PB, O_FULL], bf16, kind="ExternalInput")
    nzs_d = nc.dram_tensor("nzs", [PB, O_FULL], bf16, kind="ExternalInput")
    bias_d = nc.dram_tensor("bias", [P, O_FULL], bf16, kind="ExternalInput")
    out_d = nc.dram_tensor("out", [M_SH, O_FULL], f32, kind="ExternalOutput")

    with tile.TileContext(nc) as tc:
        with tc.tile_pool(name="fixed", bufs=1) as fixed, \
             tc.tile_pool(name="xtf", bufs=2) as xtf_pool, \
             tc.tile_pool(name="wload", bufs=2) as wload_pool, \
             tc.tile_pool(name="deq", bufs=2) as deq_pool, \
             tc.tile_pool(name="wt", bufs=2) as wt_pool, \
             tc.tile_pool(name="outp", bufs=3) as out_pool, \
             tc.tile_pool(name="psum", bufs=8, space="PSUM") as psum_pool:

            # ---- bias pre-broadcast on host: [128, O_FULL] bf16 ----
            bias_bc = fixed.tile([P, O_FULL], bf16, tag="biasbc")
            nc.sync.dma_start(bias_bc[:], bias_d[:, :])

            # ---- resident x shard: bf16 [128, N_KT, M_SH] ----
            xb = fixed.tile([P, N_KT, M_SH], bf16, tag="xb")
            for mi in range(N_MT):
                xt_f = xtf_pool.tile([P, N_KT, P], f32, tag="xtf", name="xt_f")
                nc.sync.dma_start(
                    xt_f[:],
                    xt_d[:, mi * P:(mi + 1) * P].rearrange("(t p) m -> p t m", p=P))
                nc.vector.tensor_copy(xb[:, :, mi * P:(mi + 1) * P], xt_f[:])

            # ---- o-chunk loop: stream-dequant W chunk, then matmul ----
            for oc in range(N_OC):
                osl = slice(oc * OC, (oc + 1) * OC)
                wpt_t = wload_pool.tile([P, N_V, OC], i16, tag="wpt", name="wpt_t")
                s_t = wload_pool.tile([P, N_V, OC], bf16, tag="s", name="s_t")
                nzs_t = wload_pool.tile([P, N_V, OC], bf16, tag="nzs", name="nzs_t")
                nc.sync.dma_start(
                    wpt_t[:], wpt_d[:, osl].rearrange("(v p) o -> p v o", p=P))
                nc.sync.dma_start(
                    s_t[:], sexp_d[:, osl].rearrange("(v p) o -> p v o", p=P))
                nc.sync.dma_start(
                    nzs_t[:], nzs_d[:, osl].rearrange("(v p) o -> p v o", p=P))

                WT = wt_pool.tile([P, N_KT, OC], bf16, tag="WT", name="WT")
                for t in range(N_KT):
                    u, v = t // N_V, t % N_V
                    B_t = deq_pool.tile([P, OC], i16, tag="B", name="B_t")
                    nc.vector.tensor_scalar(
                        B_t[:], wpt_t[:, v, :], u, 1,
                        mybir.AluOpType.logical_shift_right,
                        mybir.AluOpType.bitwise_and)
                    bs_t = deq_pool.tile([P, OC], bf16, tag="bs", name="bs_t")
                    nc.vector.scalar_tensor_tensor(
                        bs_t[:], B_t[:], 1.0, s_t[:, v, :],
                        mybir.AluOpType.mult, mybir.AluOpType.mult)
                    nc.vector.tensor_tensor(WT[:, t, :], bs_t[:], nzs_t[:, v, :],
                                            mybir.AluOpType.add)

                for mi in range(N_MT):
                    ps = psum_pool.tile([P, OC], f32, tag="ps", name="ps")
                    for t in range(N_KT):
                        nc.tensor.matmul(
                            ps[:], xb[:, t, mi * P:(mi + 1) * P], WT[:, t, :],
                            start=(t == 0), stop=(t == N_KT - 1))
                    out_t = out_pool.tile([P, OC], f32, tag="out", name="out_t")
                    nc.vector.tensor_tensor(out_t[:], ps[:], bias_bc[:, osl],
                                            mybir.AluOpType.add)
                    nc.sync.dma_start(out_d[mi * P:(mi + 1) * P, osl], out_t[:])
    nc.compile()
    return nc


def _get_nc(**kw):
    key = tuple(sorted(kw.items()))
    if key not in _compiled:
        _compiled[key] = _build_nc(**kw)
    return _compiled[key]


def _host_prep(x, W_packed, scale, zero, bias):
    """Layout/packing-only prep of per-core input maps."""
    import ml_dtypes
    bf16 = ml_dtypes.bfloat16
    PB = K_IN // NBITS_PER_BYTE
    NG = K_IN // GROUP_SIZE
    x = np.asarray(x, dtype=np.float32)
    W_packed = np.asarray(W_packed)
    scale2d = np.asarray(scale, dtype=np.float32).reshape(O_FULL, NG)
    zero2d = np.asarray(zero, dtype=np.float32).reshape(O_FULL, NG)
    bias = np.asarray(bias, dtype=np.float32)

    # shared (replicated) tensors
    wpt = np.ascontiguousarray(W_packed.T.astype(np.int16))          # [PB, O]
    sexp = np.ascontiguousarray(
        np.repeat(scale2d.T, NBITS_PER_BYTE, axis=0).astype(bf16))   # [PB, O]
    nzs = np.ascontiguousarray(
        np.repeat((-zero2d * scale2d).T, NBITS_PER_BYTE, axis=0).astype(bf16))
    bias2 = np.ascontiguousarray(
        np.broadcast_to(bias.astype(bf16)[None, :], (P, O_FULL)))

    in_maps = []
    for c in range(N_CORES):
        xs = x[c * M_SH:(c + 1) * M_SH]                              # [M_SH, K]
        # bit-plane-major permuted transpose: xt[u*PB + pb, m] = x[m, 8*pb + u]
        xt = xs.T.reshape(PB, NBITS_PER_BYTE, M_SH)
        xt = np.ascontiguousarray(xt.transpose(1, 0, 2).reshape(K_IN, M_SH))
        in_maps.append(dict(xt=xt, wpt=wpt, sexp=sexp, nzs=nzs, bias=bias2))
    return in_maps


def run_sharded(x, W_packed, scale, zero, bias, trace=False, **run_kwargs):
    """Compile (cached), run on 8 cores, return (full_out, BassKernelResults)."""
    from concourse.bass_utils import run_bass_kernel_spmd

    nc = _get_nc()
    in_maps = _host_prep(x, W_packed, scale, zero, bias)
    res = run_bass_kernel_spmd(nc, in_maps, core_ids=list(range(N_CORES)),
                               trace=trace, **run_kwargs)
    out = np.empty((M_FULL, O_FULL), dtype=np.float32)
    for c in range(N_CORES):
        out[c * M_SH:(c + 1) * M_SH, :] = res.results[c]["out"]
    return out, res


def kernel(x, W_packed, scale, zero, bias):
    out, _ = run_sharded(x, W_packed, scale, zero, bias)
    return out
